# revision 21
# baseline (speedup 1.0000x reference)
"""Trainium2 Bass kernel for nn_Encoder_ATTENTION (gnn_message_passing).

Math (per (b, n)):
  wn     = normalize(w_r_weight[rid[b,n]])            (table prep, host)
  d      = <e[b,n,:], wn>
  e_tr   = e - d * wn                                  (unmasked; mask folded into coeffs)
  h      = tanh(W @ [z_q[b]; e_tr] + bias)             (z-part via zw stage slices, on-chip)
  alpha  = u_a . h + u_a_b
  E      = exp(alpha) * (rid < CNT_E)
  attn   = E / sum_n(E) + rw                           (softmax w/o max-sub; logits are small)
  out[b] = sum_n (attn * mask) * e_tr

Sharding: data-parallel over batch, 512 batch rows per core x 8 cores.
Host does layout-only prep (transposes/padding/casting of weights + index
tensors); all batch-dependent math runs on device.

v2 scheme vs baseline:
  - bf16 data path (e, wn, W, zw, etr, h); f32 for dot accumulators,
    softmax denominators, and all PSUM tiles.
  - one batched indirect-DMA gather per 8 tiles instead of 8.
  - etr transposed by the DMA xbar (dma_start_transpose), one DMA per
    half-batch: no PE transposes, no PSUM roundtrip, no copy.
  - fused tensor_tensor_reduce for the projection dot and the u-dot.
  - batch-level software pipelining: projection (DVE) for batch bt runs
    while the MLP/attention stage (PE/Act/DVE) consumes batch bt-1,
    keeping the PE continuously busy (p-state ramp).
"""

import sys


def _ensure_path():
    for p in ("/opt/trn_rl_repo", "/root/.axon_site/_ro/trn_rl_repo"):
        if p not in sys.path:
            sys.path.append(p)


_ensure_path()

from contextlib import ExitStack

import ml_dtypes
import numpy as np

import concourse.bacc as bacc
import concourse.bass as bass
import concourse.tile as tile
from concourse import mybir
from concourse.bass import IndirectOffsetOnAxis
from concourse.masks import make_identity

B, NB, DIM = 4096, 64, 256
H = 2 * DIM
NCORES = 8
BC = B // NCORES            # 512 batch rows per core
ROWS = BC * NB              # 32768 (b, n) rows per core
NT = ROWS // 128            # 256 tiles of 128 rows
TPB = 8                     # tiles per batch (softmax/output batching)
NBATCH = NT // TPB          # 32
CNT_E = 1000                # padding relation id
N_WR = CNT_E + 1            # 1001 rows in w_r table
N_ZQ = CNT_E                # 1000 rows in zq table
WN_PAD = 1024               # padded wn table rows

f32 = mybir.dt.float32
bf16 = mybir.dt.bfloat16
i32 = mybir.dt.int32
AF = mybir.ActivationFunctionType
OP = mybir.AluOpType
bfnp = ml_dtypes.bfloat16


def build_nc(nbatch=NBATCH):
    nc = bacc.Bacc("TRN2")

    e_d = nc.dram_tensor("e", [ROWS, DIM], bf16, kind="ExternalInput")
    ridT_d = nc.dram_tensor("ridT", [128, NT], i32, kind="ExternalInput")
    rid16_d = nc.dram_tensor("rid16", [128, NBATCH * 64], mybir.dt.int16, kind="ExternalInput")
    rwT_d = nc.dram_tensor("rwT", [128, NT], f32, kind="ExternalInput")
    qoff_d = nc.dram_tensor("qoff", [128, BC // 128], i32, kind="ExternalInput")
    wn_d = nc.dram_tensor("wn", [WN_PAD, DIM], bf16, kind="ExternalInput")
    zq_d = nc.dram_tensor("zq", [N_ZQ, DIM], bf16, kind="ExternalInput")
    WT_d = nc.dram_tensor("WT", [128, 4, H], bf16, kind="ExternalInput")
    bias_d = nc.dram_tensor("bias", [1, H], f32, kind="ExternalInput")
    ua_d = nc.dram_tensor("ua", [1, H], f32, kind="ExternalInput")
    uab_d = nc.dram_tensor("uab", [1, 1], f32, kind="ExternalInput")
    out_d = nc.dram_tensor("out", [BC, DIM], f32, kind="ExternalOutput")
    zws_d = nc.dram_tensor("zwScratch", [BC, H], bf16, kind="Internal")

    with tile.TileContext(nc) as tc, ExitStack() as ctx:
        const = ctx.enter_context(tc.tile_pool(name="const", bufs=1))
        epool = ctx.enter_context(tc.tile_pool(name="epool", bufs=2))
        gpool = ctx.enter_context(tc.tile_pool(name="gpool", bufs=2))
        wpool = ctx.enter_context(tc.tile_pool(name="wpool", bufs=4))
        etrp = ctx.enter_context(tc.tile_pool(name="etrp", bufs=4))
        etp = ctx.enter_context(tc.tile_pool(name="etp", bufs=3))
        hpool = ctx.enter_context(tc.tile_pool(name="hpool", bufs=4))
        scp = ctx.enter_context(tc.tile_pool(name="scp", bufs=3))
        abp = ctx.enter_context(tc.tile_pool(name="abp", bufs=8))
        czp = ctx.enter_context(tc.tile_pool(name="czp", bufs=2))
        osp = ctx.enter_context(tc.tile_pool(name="osp", bufs=2))
        rsp = ctx.enter_context(tc.tile_pool(name="rsp", bufs=4))
        stgp = ctx.enter_context(tc.tile_pool(name="stgp", bufs=4))

        hps = ctx.enter_context(tc.tile_pool(name="hps", bufs=4, space="PSUM"))
        ops_ = ctx.enter_context(tc.tile_pool(name="ops", bufs=2, space="PSUM"))
        sps = ctx.enter_context(tc.tile_pool(name="sps", bufs=1, space="PSUM"))
        rbcp = ctx.enter_context(tc.tile_pool(name="rbcp", bufs=1, space="PSUM"))

        # ---------- constants ----------
        # blkpat[p, g] = 1.0 if p // 64 == g else 0.0          [128, 2]
        io2 = const.tile([128, 2], i32)
        nc.gpsimd.iota(io2[:], pattern=[[-64, 2]], base=0, channel_multiplier=1)
        bp0 = const.tile([128, 2], f32)
        bp1 = const.tile([128, 2], f32)
        nc.vector.tensor_scalar(out=bp0[:], in0=io2[:], scalar1=0, scalar2=None, op0=OP.is_ge)
        nc.vector.tensor_scalar(out=bp1[:], in0=io2[:], scalar1=63, scalar2=None, op0=OP.is_le)
        blkpat_f = const.tile([128, 2], f32)
        nc.vector.tensor_tensor(out=blkpat_f[:], in0=bp0[:], in1=bp1[:], op=OP.mult)
        blkpat = const.tile([128, 2], bf16)
        nc.vector.tensor_copy(blkpat[:], blkpat_f[:])

        # O2T[g, c] = 1.0 if c // 64 == g else 0.0             [2, 128]
        io3 = const.tile([2, 128], i32)
        nc.gpsimd.iota(io3[:], pattern=[[1, 128]], base=0, channel_multiplier=-64)
        ot0 = const.tile([2, 128], f32)
        ot1 = const.tile([2, 128], f32)
        nc.vector.tensor_scalar(out=ot0[:], in0=io3[:], scalar1=0, scalar2=None, op0=OP.is_ge)
        nc.vector.tensor_scalar(out=ot1[:], in0=io3[:], scalar1=63, scalar2=None, op0=OP.is_le)
        O2T = const.tile([2, 128], bf16)
        nc.vector.tensor_tensor(out=O2T[:], in0=ot0[:], in1=ot1[:], op=OP.mult)

        # ---------- broadcast / table loads ----------
        # (partition-step-0 DMA broadcast crashes the exec unit on this
        # runtime; broadcast across partitions via a PE outer product instead)
        ones1 = const.tile([1, 128], f32)
        nc.gpsimd.memset(ones1[:], 1.0)
        ua_row = const.tile([1, H], f32)
        nc.sync.dma_start(out=ua_row[:], in_=ua_d[:])
        bias_row = const.tile([1, H], f32)
        nc.sync.dma_start(out=bias_row[:], in_=bias_d[:])
        uab_row = const.tile([1, 1], f32)
        nc.sync.dma_start(out=uab_row[:], in_=uab_d[:])

        bc_ps = hps.tile([128, H], f32, tag="hps")
        nc.tensor.matmul(out=bc_ps[:], lhsT=ones1[:], rhs=ua_row[:])
        u_ab = const.tile([128, H], bf16)
        nc.scalar.copy(u_ab[:], bc_ps[:])
        bc_ps2 = hps.tile([128, H], f32, tag="hps")
        nc.tensor.matmul(out=bc_ps2[:], lhsT=ones1[:], rhs=bias_row[:])
        biasb = const.tile([128, H], f32)
        nc.scalar.copy(biasb[:], bc_ps2[:])
        bc_ps3 = hps.tile([128, H], f32, tag="hps")
        nc.tensor.matmul(out=bc_ps3[:, 0:1], lhsT=ones1[:], rhs=uab_row[:])
        uab_b = const.tile([128, 1], f32)
        nc.scalar.copy(uab_b[:], bc_ps3[:, 0:1])
        WTs = const.tile([128, 4, H], bf16)
        nc.sync.dma_start(out=WTs[:], in_=WT_d[:])
        ridTs = const.tile([128, NT], i32)
        nc.sync.dma_start(out=ridTs[:], in_=ridT_d[:])
        rid16s = const.tile([128, NBATCH * 64], mybir.dt.int16)
        nc.sync.dma_start(out=rid16s[:], in_=rid16_d[:])
        rwTs = const.tile([128, NT], f32)
        nc.sync.dma_start(out=rwTs[:], in_=rwT_d[:])
        qoffs = const.tile([128, BC // 128], i32)
        nc.sync.dma_start(out=qoffs[:], in_=qoff_d[:])

        # mask / masked rw, in tile-major layout [128, NT]
        ridTf = const.tile([128, NT], f32)
        nc.vector.tensor_copy(ridTf[:], ridTs[:])
        maskT = const.tile([128, NT], bf16)
        nc.vector.tensor_scalar(out=maskT[:], in0=ridTf[:], scalar1=float(CNT_E), scalar2=None, op0=OP.is_lt)
        rwmT = const.tile([128, NT], bf16)
        nc.vector.tensor_tensor(out=rwmT[:], in0=rwTs[:], in1=maskT[:], op=OP.mult)

        # ---------- zw table: zw[b] = W_z @ zq[q_rid[b]] + bias   [128, 4, H] ----------
        z_all = const.tile([128, BC // 128, DIM], bf16)
        for j in range(BC // 128):
            nc.gpsimd.indirect_dma_start(
                out=z_all[:, j, :],
                out_offset=None,
                in_=zq_d[:],
                in_offset=IndirectOffsetOnAxis(ap=qoffs[:, j : j + 1], axis=0),
            )
        # zT_all[p, 2j+k, r] = z_all[r, j, 128k+p] (xbar transpose, d inner-128)
        zT_all = const.tile([128, BC // 128 * 2, 128], bf16)
        nc.sync.dma_start_transpose(out=zT_all[:], in_=z_all[:])
        zw_all = const.tile([128, BC // 128, H], bf16)
        for j in range(BC // 128):
            zw_ps = hps.tile([128, H], f32, tag="hps")
            for k in range(2):
                nc.tensor.matmul(
                    out=zw_ps[:],
                    lhsT=zT_all[:, 2 * j + k, :],
                    rhs=WTs[:, k, :],
                    start=(k == 0),
                    stop=(k == 1),
                    skip_group_check=True,
                )
            nc.vector.tensor_tensor(out=zw_all[:, j, :], in0=zw_ps[:], in1=biasb[:], op=OP.add)

        # bounce zw to DRAM; per-batch restage from the pair-major view gives
        # a [2, TPB, H] tile whose per-tile [2, H] slice starts at partition 0
        # (PE operand base-partition constraint) -- one DMA per batch.
        nc.sync.dma_start(
            out=zws_d[:].rearrange("(j p) h -> p j h", p=128), in_=zw_all[:]
        )
        zw_re = zws_d[:].rearrange("(c r) h -> r c h", r=2)  # [2, NT, H]

        # ---------- main loop ----------
        e_re = e_d[:].rearrange("(t p) d -> p t d", p=128)  # [128, NT, DIM]

        def batch_head(bt):
            """DMAs for batch bt: e tiles, wn gather, zw stage rows."""
            t0 = bt * TPB
            e8 = epool.tile([128, TPB, DIM], bf16, tag="e8")
            nc.sync.dma_start(out=e8[:], in_=e_re[:, t0 : t0 + TPB, :])
            G8 = gpool.tile([128, TPB, DIM], bf16, tag="G8")
            nc.gpsimd.dma_gather(
                out_ap=G8[:], in_ap=wn_d[:],
                idxs_ap=rid16s[:, bt * 64 : (bt + 1) * 64],
                num_idxs=128 * TPB, num_idxs_reg=128 * TPB, elem_size=DIM,
            )
            stage8 = stgp.tile([2, TPB, H], bf16, tag="stage8")
            nc.sync.dma_start(out=stage8[:], in_=zw_re[:, t0 : t0 + TPB, :])
            etr8 = etrp.tile([128, TPB, DIM], bf16, tag="etr8")
            eT8 = etp.tile([128, 2 * TPB, 128], bf16, tag="eT8")
            alpha_b = abp.tile([128, TPB], f32, tag="alpha")
            return dict(e8=e8, G8=G8, stage8=stage8, etr8=etr8, eT8=eT8, alpha=alpha_b)

        def stage_proj(st, s):
            """projection for tile s of st's batch: etr = e - <e,wn>wn"""
            et = st["e8"][:, s, :]
            gt = st["G8"][:, s, :]
            X = wpool.tile([128, DIM], bf16, tag="X")
            dv = wpool.tile([128, 1], f32, tag="dv")
            nc.vector.tensor_tensor(out=X[:], in0=et, in1=gt, op=OP.mult)
            nc.vector.tensor_reduce(out=dv[:], in_=X[:], axis=mybir.AxisListType.X, op=OP.add)
            dG = wpool.tile([128, DIM], bf16, tag="dG")
            nc.vector.tensor_scalar(out=dG[:], in0=gt, scalar1=dv[:], scalar2=None, op0=OP.mult)
            nc.vector.tensor_tensor(out=st["etr8"][:, s, :], in0=et, in1=dG[:], op=OP.subtract)

        def stage_mlp(st, s):
            """MLP for tile s: h = tanh(W [z; etr] + b); alpha[:, s] = u . h"""
            eT8 = st["eT8"]
            h_ps = hps.tile([128, H], f32, tag="hps")
            nc.tensor.matmul(
                out=h_ps[:], lhsT=eT8[:, 2 * s, :], rhs=WTs[:, 2, :],
                start=True, stop=False, skip_group_check=True,
            )
            nc.tensor.matmul(
                out=h_ps[:], lhsT=eT8[:, 2 * s + 1, :], rhs=WTs[:, 3, :],
                start=False, stop=False, skip_group_check=True,
            )
            nc.tensor.matmul(
                out=h_ps[:], lhsT=O2T[:], rhs=st["stage8"][:, s, :],
                start=False, stop=True, skip_group_check=True,
            )
            h = hpool.tile([128, H], bf16, tag="h")
            nc.scalar.activation(out=h[:], in_=h_ps[:], func=AF.Tanh)
            sc = scp.tile([128, H], bf16, tag="sc")
            nc.vector.tensor_tensor(out=sc[:], in0=h[:], in1=u_ab[:], op=OP.mult)
            nc.vector.tensor_reduce(
                out=st["alpha"][:, s : s + 1], in_=sc[:], axis=mybir.AxisListType.X, op=OP.add
            )

        def batch_tail(bt, st):
            """softmax + coeffs + output reduction + store for batch bt"""
            t0 = bt * TPB
            alpha_b = st["alpha"]
            Eb = abp.tile([128, TPB], f32, tag="Eb")
            nc.scalar.activation(out=Eb[:], in_=alpha_b[:], func=AF.Exp, bias=uab_b[:, 0:1])
            Em = abp.tile([128, TPB], bf16, tag="Em")
            nc.vector.tensor_tensor(out=Em[:], in0=Eb[:], in1=maskT[:, t0 : t0 + TPB], op=OP.mult)

            s_ps = sps.tile([2, TPB], f32, tag="sps")
            nc.tensor.matmul(out=s_ps[:], lhsT=blkpat[:], rhs=Em[:])
            rS = rsp.tile([2, TPB], f32, tag="rS")
            nc.vector.reciprocal(rS[:], s_ps[:])
            rS_r = rsp.tile([2, TPB], bf16, tag="rSr")
            nc.vector.tensor_copy(rS_r[:], rS[:])
            rbc_ps = rbcp.tile([128, TPB], f32, tag="rbc")
            nc.tensor.matmul(out=rbc_ps[:], lhsT=O2T[:], rhs=rS_r[:])

            coeff = abp.tile([128, TPB], f32, tag="coeff")
            nc.vector.tensor_tensor(out=coeff[:], in0=Em[:], in1=rbc_ps[:], op=OP.mult)
            nc.vector.tensor_tensor(out=coeff[:], in0=coeff[:], in1=rwmT[:, t0 : t0 + TPB], op=OP.add)

            cz = czp.tile([128, TPB * 16], bf16, tag="cz")
            nc.vector.memset(cz[:], 0.0)
            for s in range(TPB):
                nc.vector.tensor_scalar(
                    out=cz[:, 16 * s + 2 * s : 16 * s + 2 * s + 2],
                    in0=blkpat[:],
                    scalar1=coeff[:, s : s + 1],
                    scalar2=None,
                    op0=OP.mult,
                )
            o_ps = ops_.tile([2 * TPB, DIM], f32, tag="ops")
            for s in range(TPB):
                nc.tensor.matmul(
                    out=o_ps[:],
                    lhsT=cz[:, 16 * s : 16 * (s + 1)],
                    rhs=st["etr8"][:, s, :],
                    start=(s == 0), stop=(s == TPB - 1), skip_group_check=True,
                )
            outS = osp.tile([2 * TPB, DIM], f32, tag="outS")
            nc.scalar.copy(outS[:], o_ps[:])
            nc.sync.dma_start(out=out_d[2 * TPB * bt : 2 * TPB * (bt + 1), :], in_=outS[:])

        batches = {}
        for bt in range(nbatch + 2):
            if bt < nbatch:
                batches[bt] = batch_head(bt)
            for i in range(TPB):
                if bt < nbatch:
                    st = batches[bt]
                    stage_proj(st, i)
                    if i == TPB // 2 - 1:
                        nc.sync.dma_start_transpose(
                            out=st["eT8"][:, 0:TPB, :],
                            in_=st["etr8"][:, 0 : TPB // 2, :],
                        )
                    elif i == TPB - 1:
                        nc.sync.dma_start_transpose(
                            out=st["eT8"][:, TPB : 2 * TPB, :],
                            in_=st["etr8"][:, TPB // 2 : TPB, :],
                        )
                if 0 <= bt - 1 < nbatch:
                    stage_mlp(batches[bt - 1], i)
                if bt >= 2 and i == 1:
                    batch_tail(bt - 2, batches.pop(bt - 2))

    nc.finalize()
    return nc


_NC = None


def _get_nc():
    global _NC
    if _NC is None:
        _NC = build_nc()
    return _NC


def _prep_in_maps(inputs):
    e = np.asarray(inputs["batch_nei_e_emb"], dtype=np.float32).astype(bfnp)
    rid = np.asarray(inputs["batch_nei_rid"]).astype(np.int32)
    rw = np.asarray(inputs["batch_nei_rw"], dtype=np.float32)
    qr = np.asarray(inputs["batch_q_rid"]).astype(np.int32)

    w = np.asarray(inputs["w_r_weight"], dtype=np.float32)
    nrm = np.maximum(np.linalg.norm(w, axis=1, keepdims=True), 1e-12)
    wn = np.zeros((WN_PAD, DIM), np.float32)
    wn[:N_WR] = w / nrm
    wn = wn.astype(bfnp)
    WT = np.asarray(inputs["attn_W_w"], dtype=np.float32).T  # [in=512, out=512]
    WT4 = np.ascontiguousarray(WT.reshape(4, 128, H).transpose(1, 0, 2)).astype(bfnp)
    zq = np.ascontiguousarray(np.asarray(inputs["zq_weight"], dtype=np.float32)).astype(bfnp)
    bias = np.asarray(inputs["attn_W_b"], dtype=np.float32).reshape(1, H)
    ua = np.asarray(inputs["u_a_w"], dtype=np.float32).reshape(1, H)
    uab = np.asarray(inputs["u_a_b"], dtype=np.float32).reshape(1, 1)

    in_maps = []
    for c in range(NCORES):
        sl = slice(BC * c, BC * (c + 1))
        ec = np.ascontiguousarray(e[sl].reshape(ROWS, DIM))
        ridc = rid[sl].reshape(ROWS)
        rwc = rw[sl].reshape(ROWS)
        qc = qr[sl]
        r16 = ridc.astype(np.int16).reshape(NBATCH, 64, 16).transpose(0, 2, 1)
        r16 = np.tile(r16, (1, 8, 1))              # [NBATCH, 128, 64]
        r16 = np.ascontiguousarray(r16.transpose(1, 0, 2).reshape(128, NBATCH * 64))
        in_maps.append(
            {
                "e": ec,
                "ridT": np.ascontiguousarray(ridc.reshape(NT, 128).T),
                "rid16": r16,
                "rwT": np.ascontiguousarray(rwc.reshape(NT, 128).T),
                "qoff": np.ascontiguousarray(qc.reshape(BC // 128, 128).T),
                "wn": wn,
                "zq": zq,
                "WT": WT4,
                "bias": bias,
                "ua": ua,
                "uab": uab,
            }
        )
    return in_maps


def run_cores(inputs, trace=False, tmpdir=None):
    from concourse.bass_utils import run_bass_kernel_spmd

    nc = _get_nc()
    in_maps = _prep_in_maps(inputs)
    res = run_bass_kernel_spmd(
        nc, in_maps, core_ids=list(range(NCORES)), trace=trace, tmpdir=tmpdir
    )
    out = np.concatenate([res.results[c]["out"] for c in range(NCORES)], axis=0)
    return out, res


def kernel(**inputs):
    out, _ = run_cores(inputs, trace=False)
    return out


# revision 23
# speedup vs baseline: 1.0860x; 1.0860x over previous
"""Trainium2 Bass kernel for nn_Encoder_ATTENTION (gnn_message_passing).

Math (per (b, n)):
  wn     = normalize(w_r_weight[rid[b,n]])            (table prep, host)
  d      = <e[b,n,:], wn>
  e_tr   = e - d * wn                                  (unmasked; mask folded into coeffs)
  h      = tanh(W @ [z_q[b]; e_tr] + bias)             (z-part via zw stage slices, on-chip)
  alpha  = u_a . h + u_a_b
  E      = exp(alpha) * (rid < CNT_E)
  attn   = E / sum_n(E) + rw                           (softmax w/o max-sub; logits are small)
  out[b] = sum_n (attn * mask) * e_tr

Sharding: data-parallel over batch, 512 batch rows per core x 8 cores.
Host does layout-only prep (transposes/padding/casting of weights + index
tensors); all batch-dependent math runs on device.

v2 scheme vs baseline:
  - bf16 data path (e, wn, W, zw, etr, h); f32 for dot accumulators,
    softmax denominators, and all PSUM tiles.
  - one batched indirect-DMA gather per 8 tiles instead of 8.
  - etr transposed by the DMA xbar (dma_start_transpose), one DMA per
    half-batch: no PE transposes, no PSUM roundtrip, no copy.
  - fused tensor_tensor_reduce for the projection dot and the u-dot.
  - batch-level software pipelining: projection (DVE) for batch bt runs
    while the MLP/attention stage (PE/Act/DVE) consumes batch bt-1,
    keeping the PE continuously busy (p-state ramp).
"""

import sys


def _ensure_path():
    for p in ("/opt/trn_rl_repo", "/root/.axon_site/_ro/trn_rl_repo"):
        if p not in sys.path:
            sys.path.append(p)


_ensure_path()

from contextlib import ExitStack

import ml_dtypes
import numpy as np

import concourse.bacc as bacc
import concourse.bass as bass
import concourse.tile as tile
from concourse import mybir
from concourse.bass import IndirectOffsetOnAxis
from concourse.masks import make_identity

B, NB, DIM = 4096, 64, 256
H = 2 * DIM
NCORES = 8
BC = B // NCORES            # 512 batch rows per core
ROWS = BC * NB              # 32768 (b, n) rows per core
NT = ROWS // 128            # 256 tiles of 128 rows
TPB = 8                     # tiles per batch (softmax/output batching)
NBATCH = NT // TPB          # 32
CNT_E = 1000                # padding relation id
N_WR = CNT_E + 1            # 1001 rows in w_r table
N_ZQ = CNT_E                # 1000 rows in zq table
WN_PAD = 1024               # padded wn table rows

f32 = mybir.dt.float32
bf16 = mybir.dt.bfloat16
i32 = mybir.dt.int32
AF = mybir.ActivationFunctionType
OP = mybir.AluOpType
bfnp = ml_dtypes.bfloat16


def build_nc(nbatch=NBATCH):
    nc = bacc.Bacc("TRN2")

    e_d = nc.dram_tensor("e", [ROWS, DIM], bf16, kind="ExternalInput")
    ridT_d = nc.dram_tensor("ridT", [128, NT], i32, kind="ExternalInput")
    rid16_d = nc.dram_tensor("rid16", [128, NBATCH * 64], mybir.dt.int16, kind="ExternalInput")
    rwT_d = nc.dram_tensor("rwT", [128, NT], f32, kind="ExternalInput")
    qoff_d = nc.dram_tensor("qoff", [128, BC // 128], i32, kind="ExternalInput")
    wn_d = nc.dram_tensor("wn", [WN_PAD, DIM], bf16, kind="ExternalInput")
    zq_d = nc.dram_tensor("zq", [N_ZQ, DIM], bf16, kind="ExternalInput")
    WT_d = nc.dram_tensor("WT", [128, 4, H], bf16, kind="ExternalInput")
    bias_d = nc.dram_tensor("bias", [1, H], f32, kind="ExternalInput")
    ua_d = nc.dram_tensor("ua", [1, H], f32, kind="ExternalInput")
    uab_d = nc.dram_tensor("uab", [1, 1], f32, kind="ExternalInput")
    out_d = nc.dram_tensor("out", [BC, DIM], f32, kind="ExternalOutput")

    with tile.TileContext(nc) as tc, ExitStack() as ctx:
        const = ctx.enter_context(tc.tile_pool(name="const", bufs=1))
        epool = ctx.enter_context(tc.tile_pool(name="epool", bufs=2))
        gpool = ctx.enter_context(tc.tile_pool(name="gpool", bufs=2))
        wpool = ctx.enter_context(tc.tile_pool(name="wpool", bufs=6))
        etrp = ctx.enter_context(tc.tile_pool(name="etrp", bufs=3))
        etp = ctx.enter_context(tc.tile_pool(name="etp", bufs=2))
        hpool = ctx.enter_context(tc.tile_pool(name="hpool", bufs=4))
        scp = ctx.enter_context(tc.tile_pool(name="scp", bufs=3))
        abp = ctx.enter_context(tc.tile_pool(name="abp", bufs=8))
        czp = ctx.enter_context(tc.tile_pool(name="czp", bufs=2))
        osp = ctx.enter_context(tc.tile_pool(name="osp", bufs=2))
        rsp = ctx.enter_context(tc.tile_pool(name="rsp", bufs=4))
        stgp = ctx.enter_context(tc.tile_pool(name="stgp", bufs=3))

        hps = ctx.enter_context(tc.tile_pool(name="hps", bufs=4, space="PSUM"))
        ops_ = ctx.enter_context(tc.tile_pool(name="ops", bufs=2, space="PSUM"))
        sps = ctx.enter_context(tc.tile_pool(name="sps", bufs=1, space="PSUM"))
        rbcp = ctx.enter_context(tc.tile_pool(name="rbcp", bufs=1, space="PSUM"))

        # ---------- constants ----------
        # blkpat[p, g] = 1.0 if p // 64 == g else 0.0          [128, 2]
        io2 = const.tile([128, 2], i32)
        nc.gpsimd.iota(io2[:], pattern=[[-64, 2]], base=0, channel_multiplier=1)
        bp0 = const.tile([128, 2], f32)
        bp1 = const.tile([128, 2], f32)
        nc.vector.tensor_scalar(out=bp0[:], in0=io2[:], scalar1=0, scalar2=None, op0=OP.is_ge)
        nc.vector.tensor_scalar(out=bp1[:], in0=io2[:], scalar1=63, scalar2=None, op0=OP.is_le)
        blkpat_f = const.tile([128, 2], f32)
        nc.vector.tensor_tensor(out=blkpat_f[:], in0=bp0[:], in1=bp1[:], op=OP.mult)
        blkpat = const.tile([128, 2], bf16)
        nc.vector.tensor_copy(blkpat[:], blkpat_f[:])

        # O2T[g, c] = 1.0 if c // 64 == g else 0.0             [2, 128]
        io3 = const.tile([2, 128], i32)
        nc.gpsimd.iota(io3[:], pattern=[[1, 128]], base=0, channel_multiplier=-64)
        ot0 = const.tile([2, 128], f32)
        ot1 = const.tile([2, 128], f32)
        nc.vector.tensor_scalar(out=ot0[:], in0=io3[:], scalar1=0, scalar2=None, op0=OP.is_ge)
        nc.vector.tensor_scalar(out=ot1[:], in0=io3[:], scalar1=63, scalar2=None, op0=OP.is_le)
        O2T = const.tile([2, 128], bf16)
        nc.vector.tensor_tensor(out=O2T[:], in0=ot0[:], in1=ot1[:], op=OP.mult)

        # ---------- broadcast / table loads ----------
        # (partition-step-0 DMA broadcast crashes the exec unit on this
        # runtime; broadcast across partitions via a PE outer product instead)
        ones1 = const.tile([1, 128], f32)
        nc.gpsimd.memset(ones1[:], 1.0)
        ua_row = const.tile([1, H], f32)
        nc.sync.dma_start(out=ua_row[:], in_=ua_d[:])
        bias_row = const.tile([1, H], f32)
        nc.sync.dma_start(out=bias_row[:], in_=bias_d[:])
        uab_row = const.tile([1, 1], f32)
        nc.sync.dma_start(out=uab_row[:], in_=uab_d[:])

        bc_ps = hps.tile([128, H], f32, tag="hps")
        nc.tensor.matmul(out=bc_ps[:], lhsT=ones1[:], rhs=ua_row[:])
        u_ab = const.tile([128, H], bf16)
        nc.scalar.copy(u_ab[:], bc_ps[:])
        bc_ps2 = hps.tile([128, H], f32, tag="hps")
        nc.tensor.matmul(out=bc_ps2[:], lhsT=ones1[:], rhs=bias_row[:])
        biasb = const.tile([128, H], f32)
        nc.scalar.copy(biasb[:], bc_ps2[:])
        bc_ps3 = hps.tile([128, H], f32, tag="hps")
        nc.tensor.matmul(out=bc_ps3[:, 0:1], lhsT=ones1[:], rhs=uab_row[:])
        uab_b = const.tile([128, 1], f32)
        nc.scalar.copy(uab_b[:], bc_ps3[:, 0:1])
        WTs = const.tile([128, 4, H], bf16)
        nc.sync.dma_start(out=WTs[:], in_=WT_d[:])
        ridTs = const.tile([128, NT], i32)
        nc.sync.dma_start(out=ridTs[:], in_=ridT_d[:])
        rid16s = const.tile([128, NBATCH * 64], mybir.dt.int16)
        nc.sync.dma_start(out=rid16s[:], in_=rid16_d[:])
        rwTs = const.tile([128, NT], f32)
        nc.sync.dma_start(out=rwTs[:], in_=rwT_d[:])
        qoffs = const.tile([128, BC // 128], i32)
        nc.sync.dma_start(out=qoffs[:], in_=qoff_d[:])

        # mask / masked rw, in tile-major layout [128, NT]
        ridTf = const.tile([128, NT], f32)
        nc.vector.tensor_copy(ridTf[:], ridTs[:])
        maskT = const.tile([128, NT], bf16)
        nc.vector.tensor_scalar(out=maskT[:], in0=ridTf[:], scalar1=float(CNT_E), scalar2=None, op0=OP.is_lt)
        rwmT = const.tile([128, NT], bf16)
        nc.vector.tensor_tensor(out=rwmT[:], in0=rwTs[:], in1=maskT[:], op=OP.mult)

        # ---------- zw table: zw[b] = W_z @ zq[q_rid[b]] + bias   [128, 4, H] ----------
        z_all = const.tile([128, BC // 128, DIM], bf16)
        for j in range(BC // 128):
            nc.gpsimd.indirect_dma_start(
                out=z_all[:, j, :],
                out_offset=None,
                in_=zq_d[:],
                in_offset=IndirectOffsetOnAxis(ap=qoffs[:, j : j + 1], axis=0),
            )
        # zT_all[p, 2j+k, r] = z_all[r, j, 128k+p] (xbar transpose, d inner-128)
        zT_all = const.tile([128, BC // 128 * 2, 128], bf16)
        nc.sync.dma_start_transpose(out=zT_all[:], in_=z_all[:])
        zw_all = const.tile([128, BC // 128, H], bf16)
        for j in range(BC // 128):
            zw_ps = hps.tile([128, H], f32, tag="hps")
            for k in range(2):
                nc.tensor.matmul(
                    out=zw_ps[:],
                    lhsT=zT_all[:, 2 * j + k, :],
                    rhs=WTs[:, k, :],
                    start=(k == 0),
                    stop=(k == 1),
                    skip_group_check=True,
                )
            nc.vector.tensor_tensor(out=zw_all[:, j, :], in0=zw_ps[:], in1=biasb[:], op=OP.add)

        # ---------- main loop ----------
        e_re = e_d[:].rearrange("(t p) d -> p t d", p=128)  # [128, NT, DIM]

        def batch_head(bt):
            """DMAs for batch bt: e tiles, wn gather, zw stage rows."""
            t0 = bt * TPB
            e8 = epool.tile([128, TPB, DIM], bf16, tag="e8")
            nc.sync.dma_start(out=e8[:], in_=e_re[:, t0 : t0 + TPB, :])
            G8 = gpool.tile([128, TPB, DIM], bf16, tag="G8")
            nc.gpsimd.dma_gather(
                out_ap=G8[:], in_ap=wn_d[:],
                idxs_ap=rid16s[:, bt * 64 : (bt + 1) * 64],
                num_idxs=128 * TPB, num_idxs_reg=128 * TPB, elem_size=DIM,
            )
            stage8 = stgp.tile([2, TPB, H], bf16, tag="stage8")
            for s in range(TPB):
                b0 = 2 * (t0 + s)
                nc.sync.dma_start(
                    out=stage8[:, s, :],
                    in_=zw_all[b0 % 128 : b0 % 128 + 2, b0 // 128, :],
                )
            etr8 = etrp.tile([128, TPB, DIM], bf16, tag="etr8")
            eT8 = etp.tile([128, 2 * TPB, 128], bf16, tag="eT8")
            alpha_b = abp.tile([128, TPB], f32, tag="alpha")
            return dict(e8=e8, G8=G8, stage8=stage8, etr8=etr8, eT8=eT8, alpha=alpha_b)

        def stage_projA(st, s):
            """X = e*wn; dv = sum(X) on the Act engine (accum reduce)"""
            et = st["e8"][:, s, :]
            gt = st["G8"][:, s, :]
            X = wpool.tile([128, DIM], bf16, tag="X")
            Xd = wpool.tile([128, DIM], bf16, tag="Xd")
            dv = wpool.tile([128, 1], f32, tag="dv")
            nc.vector.tensor_tensor(out=X[:], in0=et, in1=gt, op=OP.mult)
            nc.scalar.activation(out=Xd[:], in_=X[:], func=AF.Copy, accum_out=dv[:])
            st.setdefault("dv", {})[s] = dv

        def stage_projB(st, s):
            """etr = e - dv*wn (one tile behind projA to hide the Act hop)"""
            et = st["e8"][:, s, :]
            gt = st["G8"][:, s, :]
            dv = st["dv"].pop(s)
            dG = wpool.tile([128, DIM], bf16, tag="dG")
            nc.vector.tensor_scalar(out=dG[:], in0=gt, scalar1=dv[:], scalar2=None, op0=OP.mult)
            nc.vector.tensor_tensor(out=st["etr8"][:, s, :], in0=et, in1=dG[:], op=OP.subtract)

        def stage_mlp(st, s):
            """MLP for tile s: h = tanh(W [z; etr] + b); alpha[:, s] = u . h"""
            eT8 = st["eT8"]
            h_ps = hps.tile([128, H], f32, tag="hps")
            nc.tensor.matmul(
                out=h_ps[:], lhsT=eT8[:, 2 * s, :], rhs=WTs[:, 2, :],
                start=True, stop=False, skip_group_check=True,
            )
            nc.tensor.matmul(
                out=h_ps[:], lhsT=eT8[:, 2 * s + 1, :], rhs=WTs[:, 3, :],
                start=False, stop=False, skip_group_check=True,
            )
            nc.tensor.matmul(
                out=h_ps[:], lhsT=O2T[:], rhs=st["stage8"][:, s, :],
                start=False, stop=True, skip_group_check=True,
            )
            h = hpool.tile([128, H], bf16, tag="h")
            nc.scalar.activation(out=h[:], in_=h_ps[:], func=AF.Tanh)
            sc = scp.tile([128, H], bf16, tag="sc")
            nc.vector.tensor_tensor(out=sc[:], in0=h[:], in1=u_ab[:], op=OP.mult)
            nc.vector.tensor_reduce(
                out=st["alpha"][:, s : s + 1], in_=sc[:], axis=mybir.AxisListType.X, op=OP.add
            )

        def batch_tail(bt, st):
            """softmax + coeffs + output reduction + store for batch bt"""
            t0 = bt * TPB
            alpha_b = st["alpha"]
            Eb = abp.tile([128, TPB], f32, tag="Eb")
            nc.scalar.activation(out=Eb[:], in_=alpha_b[:], func=AF.Exp, bias=uab_b[:, 0:1])
            Em = abp.tile([128, TPB], bf16, tag="Em")
            nc.vector.tensor_tensor(out=Em[:], in0=Eb[:], in1=maskT[:, t0 : t0 + TPB], op=OP.mult)

            s_ps = sps.tile([2, TPB], f32, tag="sps")
            nc.tensor.matmul(out=s_ps[:], lhsT=blkpat[:], rhs=Em[:])
            rS = rsp.tile([2, TPB], f32, tag="rS")
            nc.vector.reciprocal(rS[:], s_ps[:])
            rS_r = rsp.tile([2, TPB], bf16, tag="rSr")
            nc.vector.tensor_copy(rS_r[:], rS[:])
            rbc_ps = rbcp.tile([128, TPB], f32, tag="rbc")
            nc.tensor.matmul(out=rbc_ps[:], lhsT=O2T[:], rhs=rS_r[:])

            coeff = abp.tile([128, TPB], f32, tag="coeff")
            nc.vector.tensor_tensor(out=coeff[:], in0=Em[:], in1=rbc_ps[:], op=OP.mult)
            nc.vector.tensor_tensor(out=coeff[:], in0=coeff[:], in1=rwmT[:, t0 : t0 + TPB], op=OP.add)

            cz = czp.tile([128, TPB * 16], bf16, tag="cz")
            nc.gpsimd.memset(cz[:], 0.0)
            for s in range(TPB):
                nc.vector.tensor_scalar(
                    out=cz[:, 16 * s + 2 * s : 16 * s + 2 * s + 2],
                    in0=blkpat[:],
                    scalar1=coeff[:, s : s + 1],
                    scalar2=None,
                    op0=OP.mult,
                )
            o_ps = ops_.tile([2 * TPB, DIM], f32, tag="ops")
            for s in range(TPB):
                nc.tensor.matmul(
                    out=o_ps[:],
                    lhsT=cz[:, 16 * s : 16 * (s + 1)],
                    rhs=st["etr8"][:, s, :],
                    start=(s == 0), stop=(s == TPB - 1), skip_group_check=True,
                )
            outS = osp.tile([2 * TPB, DIM], f32, tag="outS")
            nc.scalar.copy(outS[:], o_ps[:])
            nc.sync.dma_start(out=out_d[2 * TPB * bt : 2 * TPB * (bt + 1), :], in_=outS[:])

        batches = {}
        for bt in range(nbatch + 1):
            if bt < nbatch:
                batches[bt] = batch_head(bt)
            for i in range(TPB + 1):
                if bt < nbatch:
                    st = batches[bt]
                    if i < TPB:
                        stage_projA(st, i)
                    if i >= 1:
                        stage_projB(st, i - 1)
                    if i - 1 == TPB // 2 - 1:
                        nc.sync.dma_start_transpose(
                            out=st["eT8"][:, 0:TPB, :],
                            in_=st["etr8"][:, 0 : TPB // 2, :],
                        )
                    elif i - 1 == TPB - 1:
                        nc.sync.dma_start_transpose(
                            out=st["eT8"][:, TPB : 2 * TPB, :],
                            in_=st["etr8"][:, TPB // 2 : TPB, :],
                        )
                if bt > 0 and i < TPB:
                    stage_mlp(batches[bt - 1], i)
            if bt > 0:
                batch_tail(bt - 1, batches.pop(bt - 1))

    nc.finalize()
    return nc


_NC = None


def _get_nc():
    global _NC
    if _NC is None:
        _NC = build_nc()
    return _NC


def _prep_in_maps(inputs):
    e = np.asarray(inputs["batch_nei_e_emb"], dtype=np.float32).astype(bfnp)
    rid = np.asarray(inputs["batch_nei_rid"]).astype(np.int32)
    rw = np.asarray(inputs["batch_nei_rw"], dtype=np.float32)
    qr = np.asarray(inputs["batch_q_rid"]).astype(np.int32)

    w = np.asarray(inputs["w_r_weight"], dtype=np.float32)
    nrm = np.maximum(np.linalg.norm(w, axis=1, keepdims=True), 1e-12)
    wn = np.zeros((WN_PAD, DIM), np.float32)
    wn[:N_WR] = w / nrm
    wn = wn.astype(bfnp)
    WT = np.asarray(inputs["attn_W_w"], dtype=np.float32).T  # [in=512, out=512]
    WT4 = np.ascontiguousarray(WT.reshape(4, 128, H).transpose(1, 0, 2)).astype(bfnp)
    zq = np.ascontiguousarray(np.asarray(inputs["zq_weight"], dtype=np.float32)).astype(bfnp)
    bias = np.asarray(inputs["attn_W_b"], dtype=np.float32).reshape(1, H)
    ua = np.asarray(inputs["u_a_w"], dtype=np.float32).reshape(1, H)
    uab = np.asarray(inputs["u_a_b"], dtype=np.float32).reshape(1, 1)

    in_maps = []
    for c in range(NCORES):
        sl = slice(BC * c, BC * (c + 1))
        ec = np.ascontiguousarray(e[sl].reshape(ROWS, DIM))
        ridc = rid[sl].reshape(ROWS)
        rwc = rw[sl].reshape(ROWS)
        qc = qr[sl]
        r16 = ridc.astype(np.int16).reshape(NBATCH, 64, 16).transpose(0, 2, 1)
        r16 = np.tile(r16, (1, 8, 1))              # [NBATCH, 128, 64]
        r16 = np.ascontiguousarray(r16.transpose(1, 0, 2).reshape(128, NBATCH * 64))
        in_maps.append(
            {
                "e": ec,
                "ridT": np.ascontiguousarray(ridc.reshape(NT, 128).T),
                "rid16": r16,
                "rwT": np.ascontiguousarray(rwc.reshape(NT, 128).T),
                "qoff": np.ascontiguousarray(qc.reshape(BC // 128, 128).T),
                "wn": wn,
                "zq": zq,
                "WT": WT4,
                "bias": bias,
                "ua": ua,
                "uab": uab,
            }
        )
    return in_maps


def run_cores(inputs, trace=False, tmpdir=None):
    from concourse.bass_utils import run_bass_kernel_spmd

    nc = _get_nc()
    in_maps = _prep_in_maps(inputs)
    res = run_bass_kernel_spmd(
        nc, in_maps, core_ids=list(range(NCORES)), trace=trace, tmpdir=tmpdir
    )
    out = np.concatenate([res.results[c]["out"] for c in range(NCORES)], axis=0)
    return out, res


def kernel(**inputs):
    out, _ = run_cores(inputs, trace=False)
    return out
